# revision 1
# baseline (speedup 1.0000x reference)
"""ChessformerAttention Trainium2 kernel.

Full-input contract: kernel(**inputs) takes the unsharded inputs
(x [256,64,1024] f32, bias [1,16,64,64] f32, Wq/Wk/Wv/Wo [1024,1024] f32)
and returns the full [256,64,1024] f32 output.

Strategy: data-parallel over batch across 8 NeuronCores (32 batches each).
On-device pipeline per core (all matmuls in bf16, f32 accumulation):
  xT = transpose(cast(x))                      (PE transposes)
  qT = Wq^T-form proj, kT likewise             ([head_dim, tokens] layout)
  v  = x @ Wv                                  ([tokens, head_dim] layout)
  per (batch, head): scoresT = K Q^T via PE, exp on ACT, * exp(bias) on DVE,
  out = exp @ V with a parallel ones-matmul giving softmax denominators,
  per-partition reciprocal+multiply normalizes, PE transposes the result and
  a final bf16 matmul applies Wo.
Host pre-work: shard x, cast weights to bf16, precompute exp(bias) transposed
(these are input-layout transforms; all FLOPs stay on device).
"""

import os
import numpy as np
import ml_dtypes

KPHASES = os.environ.get("KPHASES", "ABCDE")
KC_SKIP = set(os.environ.get("KC_SKIP", "").split(","))

B, L, D = 256, 64, 1024
H, HD = 16, 64
N_CORES = 8
BC = B // N_CORES            # batches per core
T = BC * L                   # tokens per core
SG = 4                       # super-groups per core
TSG = T // SG                # tokens per super-group
BSG = BC // SG               # batches per super-group
P = 128
KD = D // P                  # 128-row chunks of the model dim
MSG = TSG // P               # token chunks per super-group

_compiled = None


def _build():
    import concourse.bass as bass
    import concourse.mybir as mybir
    import concourse.tile as tile
    from concourse import bacc
    from concourse.masks import make_identity
    from contextlib import ExitStack

    bf16 = mybir.dt.bfloat16
    f32 = mybir.dt.float32
    EXP = mybir.ActivationFunctionType.Exp

    nc = bacc.Bacc(
        "TRN2",
        target_bir_lowering=False,
        debug=False,
        enable_asserts=False,
        num_devices=N_CORES,
    )
    x_d = nc.dram_tensor("x", [T, D], f32, kind="ExternalInput").ap()
    w_d = {
        name: nc.dram_tensor(name, [D, D], bf16, kind="ExternalInput").ap()
        for name in ("wq", "wk", "wv", "wo")
    }
    eb_d = nc.dram_tensor("expbt", [P, H * L], f32, kind="ExternalInput").ap()
    out_d = nc.dram_tensor("out", [T, D], f32, kind="ExternalOutput").ap()

    with tile.TileContext(nc) as tc, ExitStack() as ctx:
        const = ctx.enter_context(tc.tile_pool(name="const", bufs=1))
        wpool = ctx.enter_context(tc.tile_pool(name="w", bufs=1))
        qkv = ctx.enter_context(tc.tile_pool(name="qkv", bufs=1))
        xtout = ctx.enter_context(tc.tile_pool(name="xtout", bufs=1))
        stage = ctx.enter_context(tc.tile_pool(name="stage", bufs=2))
        astage = ctx.enter_context(tc.tile_pool(name="astage", bufs=3))
        pmm = ctx.enter_context(tc.tile_pool(name="pmm", bufs=6, space="PSUM"))
        pden = ctx.enter_context(tc.tile_pool(name="pden", bufs=2, space="PSUM"))

        ident = const.tile([P, P], bf16, tag="ident", name="ident")
        make_identity(nc, ident[:])
        ones = const.tile([P, 1], bf16, tag="ones", name="ones")
        nc.any.memset(ones[:], 1.0)
        expbt = const.tile([P, H * L], f32, tag="expbt", name="expbt")
        nc.sync.dma_start(expbt[:], eb_d[:])

        W = {}
        for name in ("wq", "wk", "wv", "wo"):
            W[name] = []
            for k in range(KD):
                t = wpool.tile([P, D], bf16, tag=f"{name}{k}", name=f"{name}{k}")
                nc.sync.dma_start(t[:], w_d[name][k * P:(k + 1) * P, :])
                W[name].append(t)

        if "E" not in KPHASES:
            zfin = stage.tile([P, D], f32, tag="fin", name="fin")
            nc.any.memset(zfin[:], 0.0)
            for mm_ in range(T // P):
                nc.sync.dma_start(out_d[mm_ * P:(mm_ + 1) * P, :], zfin[:])

        for sg in range(SG):
            t0 = sg * TSG

            # ---- phase A: load x, cast to bf16, transpose to [D, tokens] ----
            xT = [xtout.tile([P, TSG], bf16, tag=f"xT{k}", name=f"xT{k}") for k in range(KD)]
            for m in range(MSG):
                for half in range(2):
                    xs = stage.tile([P, 512], f32, tag="xstage", name="xstage")
                    nc.sync.dma_start(
                        xs[:],
                        x_d[t0 + m * P: t0 + (m + 1) * P, half * 512:(half + 1) * 512],
                    )
                    xb = stage.tile([P, 512], bf16, tag="xbf", name="xbf")
                    nc.any.tensor_copy(xb[:], xs[:])
                    for k2 in range(4):
                        k = half * 4 + k2
                        pt = pmm.tile([P, P], bf16, tag="mm", name="mm")
                        nc.tensor.transpose(pt[:], xb[:, k2 * P:(k2 + 1) * P], ident[:])
                        nc.any.tensor_copy(xT[k][:, m * P:(m + 1) * P], pt[:])

            if "B" not in KPHASES:
                continue
            # ---- phase B: q/k projections ([hn, tokens]) and v ([tokens, hn]) ----
            qT = [qkv.tile([P, TSG], bf16, tag=f"qT{n}", name=f"qT{n}") for n in range(KD)]
            kT = [qkv.tile([P, TSG], bf16, tag=f"kT{n}", name=f"kT{n}") for n in range(KD)]
            T2 = TSG // 512
            for wkey, dst in (("wq", qT), ("wk", kT)):
                for n in range(KD):
                    ps = [pmm.tile([P, 512], f32, tag="mm", name="mm") for _ in range(T2)]
                    for k in range(KD):
                        for t2 in range(T2):
                            nc.tensor.matmul(
                                ps[t2][:],
                                lhsT=W[wkey][k][:, n * P:(n + 1) * P],
                                rhs=xT[k][:, t2 * 512:(t2 + 1) * 512],
                                start=(k == 0),
                                stop=(k == KD - 1),
                            )
                    for t2 in range(T2):
                        nc.any.tensor_copy(dst[n][:, t2 * 512:(t2 + 1) * 512], ps[t2][:])

            v_sb = [qkv.tile([P, D], bf16, tag=f"v{m}", name=f"v{m}") for m in range(MSG)]
            for m in range(MSG):
                ps = [pmm.tile([P, 512], f32, tag="mm", name="mm") for _ in range(2)]
                for k in range(KD):
                    for n2 in range(2):
                        nc.tensor.matmul(
                            ps[n2][:],
                            lhsT=xT[k][:, m * P:(m + 1) * P],
                            rhs=W["wv"][k][:, n2 * 512:(n2 + 1) * 512],
                            start=(k == 0),
                            stop=(k == KD - 1),
                        )
                for n2 in range(2):
                    nc.any.tensor_copy(v_sb[m][:, n2 * 512:(n2 + 1) * 512], ps[n2][:])

            if "C" not in KPHASES:
                continue
            # ---- phase C: attention (all matmuls at partition base 0) ----
            qT_lo = [qkv.tile([64, TSG], bf16, tag=f"qTlo{n}", name=f"qTlo{n}") for n in range(KD)]
            kT_lo = [qkv.tile([64, TSG], bf16, tag=f"kTlo{n}", name=f"kTlo{n}") for n in range(KD)]
            for n in range(KD):
                nc.sync.dma_start(qT_lo[n][:], qT[n][64:128, :])
                nc.sync.dma_start(kT_lo[n][:], kT[n][64:128, :])
            v_lo = [qkv.tile([64, D], bf16, tag=f"vlo{m}", name=f"vlo{m}") for m in range(MSG)]
            for m in range(MSG):
                nc.sync.dma_start(v_lo[m][:], v_sb[m][64:128, :])

            out_all = [qkv.tile([P, D], bf16, tag=f"oall{m}", name=f"oall{m}") for m in range(MSG)]
            for bl in range(BSG):
                tok = bl * L
                m_b = tok // P
                vr = (bl % 2) * 64
                vsrc = v_sb[m_b] if vr == 0 else v_lo[m_b]
                # scoresT blocks [lk, lq] for 8 heads per PSUM bank; exp in place
                expts = []
                for oct in range(2):
                    pscore = pmm.tile([64, 512], f32, tag="mm", name="mm")
                    for j in range(8):
                        h = oct * 8 + j
                        hc, odd = h // 2, h % 2
                        kt = kT_lo[hc] if odd else kT[hc]
                        qt = qT_lo[hc] if odd else qT[hc]
                        nc.tensor.matmul(
                            pscore[:, j * 64:(j + 1) * 64],
                            lhsT=kt[0:64, tok:tok + 64],
                            rhs=qt[0:64, tok:tok + 64],
                            start=True,
                            stop=True,
                        )
                    nc.scalar.activation(pscore[:], pscore[:], EXP, scale=0.125)
                    et_b = astage.tile([64, 512], bf16, tag="expb", name="expb")
                    nc.any.tensor_mul(
                        et_b[:], pscore[:], expbt[0:64, oct * 512:(oct + 1) * 512]
                    )
                    expts.append(et_b)
                # attention-weighted V plus denominator columns
                pden_t = pden.tile([64, 16], f32, tag="den", name="den")
                pouts = []
                for oct in range(2):
                    pout = pmm.tile([64, 512], f32, tag="mm", name="mm")
                    for j in range(8):
                        h = oct * 8 + j
                        nc.tensor.matmul(
                            pout[:, j * 64:(j + 1) * 64],
                            lhsT=expts[oct][:, j * 64:(j + 1) * 64],
                            rhs=vsrc[0:64, h * 64:(h + 1) * 64],
                            start=True,
                            stop=True,
                        )
                        nc.tensor.matmul(
                            pden_t[:, h:h + 1],
                            lhsT=expts[oct][:, j * 64:(j + 1) * 64],
                            rhs=ones[0:64, :],
                            start=True,
                            stop=True,
                        )
                    pouts.append(pout)
                recip = astage.tile([64, 16], f32, tag="recip", name="recip")
                nc.vector.reciprocal(recip[:], pden_t[:])
                for oct in range(2):
                    oa = astage.tile([64, 512], bf16, tag="oa", name="oa")
                    nc.any.tensor_mul(
                        oa[:].rearrange("p (h c) -> p h c", c=64),
                        pouts[oct][:].rearrange("p (h c) -> p h c", c=64),
                        recip[:, oct * 8:(oct + 1) * 8][:, :, None].broadcast_to(
                            [64, 8, 64]
                        ),
                    )
                    nc.sync.dma_start(
                        out_all[m_b][vr:vr + 64, oct * 512:(oct + 1) * 512], oa[:]
                    )

            if "D" not in KPHASES:
                continue
            # ---- phase D: transpose attention output to [hn, tokens] ----
            outT = [xtout.tile([P, TSG], bf16, tag=f"xT{k}", name=f"xT{k}") for k in range(KD)]
            for m in range(MSG):
                for k in range(KD):
                    pt = pmm.tile([P, P], bf16, tag="mm", name="mm")
                    nc.tensor.transpose(pt[:], out_all[m][:, k * P:(k + 1) * P], ident[:])
                    nc.any.tensor_copy(outT[k][:, m * P:(m + 1) * P], pt[:])

            if "E" not in KPHASES:
                continue
            # ---- phase E: final projection ----
            for m in range(MSG):
                ps = [pmm.tile([P, 512], f32, tag="mm", name="mm") for _ in range(2)]
                for k in range(KD):
                    for n2 in range(2):
                        nc.tensor.matmul(
                            ps[n2][:],
                            lhsT=outT[k][:, m * P:(m + 1) * P],
                            rhs=W["wo"][k][:, n2 * 512:(n2 + 1) * 512],
                            start=(k == 0),
                            stop=(k == KD - 1),
                        )
                for n2 in range(2):
                    fin = stage.tile([P, 512], f32, tag="fin2", name="fin2")
                    nc.any.tensor_copy(fin[:], ps[n2][:])
                    nc.sync.dma_start(
                        out_d[t0 + m * P: t0 + (m + 1) * P, n2 * 512:(n2 + 1) * 512],
                        fin[:],
                    )

    nc.compile()
    return nc


def _get_compiled():
    global _compiled
    if _compiled is None:
        _compiled = _build()
    return _compiled


def _prep_inputs(x, bias, Wq, Wk, Wv, Wo):
    bf = ml_dtypes.bfloat16
    xr = np.ascontiguousarray(x.reshape(N_CORES, T, D))
    ws = {
        "wq": np.ascontiguousarray(Wq.astype(bf)),
        "wk": np.ascontiguousarray(Wk.astype(bf)),
        "wv": np.ascontiguousarray(Wv.astype(bf)),
        "wo": np.ascontiguousarray(Wo.astype(bf)),
    }
    eb = np.exp(bias[0].astype(np.float32))          # [h, lq, lk]
    ebt = eb.transpose(2, 0, 1).reshape(L, H * L)    # [lk, h*L + lq]
    ebt = np.ascontiguousarray(np.concatenate([ebt, ebt], axis=0))  # [128, H*L]
    in_maps = [
        {"x": xr[c], "expbt": ebt, **ws} for c in range(N_CORES)
    ]
    return in_maps


def kernel(x, bias, Wq, Wk, Wv, Wo, _trace=False, _trace_kwargs=None):
    from concourse.bass_utils import run_bass_kernel_spmd

    nc = _get_compiled()
    in_maps = _prep_inputs(
        np.asarray(x, dtype=np.float32),
        np.asarray(bias, dtype=np.float32),
        np.asarray(Wq, dtype=np.float32),
        np.asarray(Wk, dtype=np.float32),
        np.asarray(Wv, dtype=np.float32),
        np.asarray(Wo, dtype=np.float32),
    )
    res = run_bass_kernel_spmd(
        nc, in_maps, list(range(N_CORES)), trace=_trace, **(_trace_kwargs or {})
    )
    out = np.stack([np.asarray(res.results[c]["out"]) for c in range(N_CORES)])
    out = out.reshape(B, L, D).astype(np.float32)
    if _trace:
        return out, res
    return out



# revision 12
# speedup vs baseline: 1.0177x; 1.0177x over previous
"""ChessformerAttention Trainium2 kernel (v2).

Full-input contract: kernel(**inputs) takes the unsharded inputs
(x [256,64,1024] f32, bias [1,16,64,64] f32, Wq/Wk/Wv/Wo [1024,1024] f32)
and returns the full [256,64,1024] f32 output.

Strategy: data-parallel over batch across 8 NeuronCores (32 batches each).
Host pre-work (input-layout transforms only): shard x, pre-transpose to
[D, tokens] bf16, cast weights to bf16, precompute exp(bias) transposed.

On-device per core (all matmuls bf16, f32 accumulation):
  qT/kT = W^T-stationary projections -> [head_dim-major, tokens]
  v     = x @ Wv                     -> [tokens, head_dim-major]
  attention in 2-batch blocks (128 partitions: batch b0 on 0-63, b1 on
  64-127 via PE tile_position): scoresT = K Q^T, exp on ACT (128-wide),
  * exp(bias) on DVE, out = exp @ V with piggybacked ones-matmul
  denominators, reciprocal+broadcast multiply normalizes into a
  [tokens, D] tile, DMA-XBAR transposes to [D-major, tokens], and a
  final matmul applies Wo.
"""

import os
import numpy as np
import ml_dtypes

KTRANS = os.environ.get("KTRANS", "dma")   # "dma" | "pe"

B, L, D = 256, 64, 1024
H, HD = 16, 64
N_CORES = 8
BC = B // N_CORES            # batches per core
T = BC * L                   # tokens per core
SG = 4                       # super-groups per core
TSG = T // SG                # tokens per super-group
NBLK = TSG // 128            # 2-batch blocks per super-group
P = 128
KD = D // P                  # 128-row chunks of the model dim

_compiled = None


def _build():
    import concourse.bass as bass
    import concourse.mybir as mybir
    import concourse.tile as tile
    from concourse import bacc
    from contextlib import ExitStack

    from concourse.masks import make_identity

    bf16 = mybir.dt.bfloat16
    f32 = mybir.dt.float32
    EXP = mybir.ActivationFunctionType.Exp

    nc = bacc.Bacc(
        "TRN2",
        target_bir_lowering=False,
        debug=False,
        enable_asserts=False,
        num_devices=N_CORES,
    )
    xt_d = nc.dram_tensor("xt", [D, T], bf16, kind="ExternalInput").ap()
    w_d = {
        name: nc.dram_tensor(name, [D, D], bf16, kind="ExternalInput").ap()
        for name in ("wq", "wk", "wv", "wo")
    }
    eb_d = nc.dram_tensor("expbt", [P, H * L], bf16, kind="ExternalInput").ap()
    out_d = nc.dram_tensor("out", [T, D], f32, kind="ExternalOutput").ap()

    with tile.TileContext(nc) as tc, ExitStack() as ctx:
        const = ctx.enter_context(tc.tile_pool(name="const", bufs=1))
        wpool = ctx.enter_context(tc.tile_pool(name="w", bufs=1))
        xpool = ctx.enter_context(tc.tile_pool(name="x", bufs=1))
        qkv = ctx.enter_context(tc.tile_pool(name="qkv", bufs=2))
        q2p = ctx.enter_context(tc.tile_pool(name="q2", bufs=4))
        etp = ctx.enter_context(tc.tile_pool(name="et", bufs=2))
        oap = ctx.enter_context(tc.tile_pool(name="oa", bufs=2))
        otp = ctx.enter_context(tc.tile_pool(name="ot", bufs=2))
        rcp = ctx.enter_context(tc.tile_pool(name="rc", bufs=2))
        finp = ctx.enter_context(tc.tile_pool(name="fin", bufs=3))
        ppj = ctx.enter_context(tc.tile_pool(name="ppj", bufs=2, space="PSUM"))
        psc = ctx.enter_context(
            tc.tile_pool(name="psc", bufs=3 if KTRANS == "dma" else 2, space="PSUM")
        )
        pou = ctx.enter_context(tc.tile_pool(name="pou", bufs=2, space="PSUM"))
        pde = ctx.enter_context(tc.tile_pool(name="pde", bufs=1, space="PSUM"))
        if KTRANS == "pe":
            ptp = ctx.enter_context(tc.tile_pool(name="ptp", bufs=1, space="PSUM"))

        ones = const.tile([P, 1], bf16, tag="ones", name="ones")
        nc.any.memset(ones[:], 1.0)
        if KTRANS == "pe":
            ident = const.tile([P, P], bf16, tag="ident", name="ident")
            make_identity(nc, ident[:])
        expbt = const.tile([P, H * L], bf16, tag="expbt", name="expbt")
        nc.sync.dma_start(expbt[:], eb_d[:])

        W = {}
        for name in ("wq", "wk", "wv", "wo"):
            W[name] = []
            for k in range(KD):
                t = wpool.tile([P, D], bf16, tag=f"{name}{k}", name=f"{name}{k}")
                nc.sync.dma_start(t[:], w_d[name][k * P:(k + 1) * P, :])
                W[name].append(t)

        xt = []
        for k in range(KD):
            t = xpool.tile([P, T], bf16, tag=f"xt{k}", name=f"xt{k}")
            nc.sync.dma_start(t[:], xt_d[k * P:(k + 1) * P, :])
            xt.append(t)

        for sg in range(SG):
            t0 = sg * TSG

            # ---- q/k projections -> [hn-major, tokens-of-sg], one wide tile ----
            qkT = {}
            for wkey in ("wq", "wk"):
                big = qkv.tile([P, KD, TSG], bf16, tag=f"{wkey}T", name=f"{wkey}T")
                for n in range(KD):
                    ps = ppj.tile([P, TSG], f32, tag="pp", name="pp")
                    for k in range(KD):
                        nc.tensor.matmul(
                            ps[:],
                            lhsT=W[wkey][k][:, n * P:(n + 1) * P],
                            rhs=xt[k][:, t0:t0 + TSG],
                            start=(k == 0),
                            stop=(k == KD - 1),
                        )
                    if wkey == "wq":
                        nc.scalar.activation(
                            big[:, n, :], ps[:], mybir.ActivationFunctionType.Copy
                        )
                    else:
                        nc.vector.tensor_copy(big[:, n, :], ps[:])
                qkT[wkey] = big

            # ---- v projection -> [tokens, hn-major] ----
            v_sb = []
            for m in range(NBLK):
                t = qkv.tile([P, D], bf16, tag=f"v{m}", name=f"v{m}")
                for n2 in range(2):
                    ps = ppj.tile([P, 512], f32, tag="pp", name="pp")
                    for k in range(KD):
                        nc.tensor.matmul(
                            ps[:],
                            lhsT=xt[k][:, t0 + m * P:t0 + (m + 1) * P],
                            rhs=W["wv"][k][:, n2 * 512:(n2 + 1) * 512],
                            start=(k == 0),
                            stop=(k == KD - 1),
                        )
                    if n2 == 0:
                        nc.scalar.activation(
                            t[:, :512], ps[:], mybir.ActivationFunctionType.Copy
                        )
                    else:
                        nc.vector.tensor_copy(t[:, 512:], ps[:])
                v_sb.append(t)

            # ---- batch-major q/k tiles per block (diagonal tile positions) ----
            # q2/k2: [b*64 + hd, h*64 + lq]
            def rearrange_qk(blk):
                tok0 = blk * P
                out2 = []
                for wkey in ("wq", "wk"):
                    dst = q2p.tile([P, H * 64], bf16, tag=f"{wkey}2", name=f"{wkey}2")
                    src = qkT[wkey]
                    for b in range(2):
                        for par in range(2):
                            nc.sync.dma_start(
                                dst[b * 64:(b + 1) * 64, :].rearrange(
                                    "p (n x) -> p n x", x=128
                                )[:, :, par * 64:(par + 1) * 64],
                                src[par * 64:(par + 1) * 64, :,
                                    tok0 + b * 64:tok0 + (b + 1) * 64],
                            )
                    out2.append(dst)
                return out2

            outT = [
                otp.tile([P, TSG], bf16, tag=f"oT{k}", name=f"oT{k}")
                for k in range(KD)
            ]

            # ---- attention: 2-batch blocks, software-pipelined ----
            def attn_scores(blk, q2, k2):
                ets = []
                for oct in range(2):
                    ps = psc.tile([P, 512], f32, tag="ps", name="ps")
                    for j in range(8):
                        h = oct * 8 + j
                        for b in range(2):
                            nc.tensor.matmul(
                                ps[b * 64:(b + 1) * 64, j * 64:(j + 1) * 64],
                                lhsT=k2[b * 64:(b + 1) * 64, h * 64:(h + 1) * 64],
                                rhs=q2[b * 64:(b + 1) * 64, h * 64:(h + 1) * 64],
                                start=True,
                                stop=True,
                            )
                    eo = etp.tile([P, 512], bf16, tag=f"eo{oct}", name=f"eo{oct}")
                    nc.scalar.activation(eo[:], ps[:], EXP, scale=0.125)
                    et = etp.tile([P, 512], bf16, tag=f"et{oct}", name=f"et{oct}")
                    nc.vector.tensor_mul(
                        et[:], eo[:], expbt[:, oct * 512:(oct + 1) * 512]
                    )
                    ets.append(et)
                return ets

            def attn_out(blk, ets):
                tok0 = blk * P
                pd = pde.tile([P, 16], f32, tag="pd", name="pd")
                pouts = []
                for oct in range(2):
                    po = pou.tile([P, 512], f32, tag="po", name="po")
                    for j in range(8):
                        h = oct * 8 + j
                        for b in range(2):
                            sl = ets[oct][b * 64:(b + 1) * 64, j * 64:(j + 1) * 64]
                            nc.tensor.matmul(
                                po[b * 64:(b + 1) * 64, j * 64:(j + 1) * 64],
                                lhsT=sl,
                                rhs=v_sb[blk][b * 64:(b + 1) * 64, h * 64:(h + 1) * 64],
                                start=True,
                                stop=True,
                            )
                            nc.tensor.matmul(
                                pd[b * 64:(b + 1) * 64, h:h + 1],
                                lhsT=sl,
                                rhs=ones[b * 64:(b + 1) * 64, :],
                                start=True,
                                stop=True,
                                skip_group_check=True,
                            )
                    pouts.append(po)
                rc = rcp.tile([P, 16], f32, tag="rc", name="rc")
                nc.vector.reciprocal(rc[:], pd[:])
                oall = oap.tile([P, D], bf16, tag="oall", name="oall")
                for oct in range(2):
                    nc.vector.tensor_mul(
                        oall[:, oct * 512:(oct + 1) * 512].rearrange(
                            "p (h c) -> p h c", c=64
                        ),
                        pouts[oct][:].rearrange("p (h c) -> p h c", c=64),
                        rc[:, oct * 8:(oct + 1) * 8][:, :, None].broadcast_to(
                            [P, 8, 64]
                        ),
                    )
                if KTRANS == "dma":
                    for k in range(KD):
                        nc.sync.dma_start_transpose(
                            outT[k][:, tok0:tok0 + P], oall[:, k * P:(k + 1) * P]
                        )
                else:
                    for half in range(2):
                        pt = ptp.tile([P, 512], bf16, tag="pt", name="pt")
                        for k2 in range(4):
                            k = half * 4 + k2
                            nc.tensor.transpose(
                                pt[:, k2 * P:(k2 + 1) * P],
                                oall[:, k * P:(k + 1) * P],
                                ident[:],
                            )
                            if k2 % 2 == 0:
                                nc.scalar.activation(
                                    outT[k][:, tok0:tok0 + P],
                                    pt[:, k2 * P:(k2 + 1) * P],
                                    mybir.ActivationFunctionType.Copy,
                                )
                            else:
                                nc.vector.tensor_copy(
                                    outT[k][:, tok0:tok0 + P],
                                    pt[:, k2 * P:(k2 + 1) * P],
                                )

            q2k2 = [rearrange_qk(blk) for blk in range(NBLK)]

            ets_cur = attn_scores(0, *q2k2[0])
            for blk in range(NBLK):
                ets_next = (
                    attn_scores(blk + 1, *q2k2[blk + 1]) if blk + 1 < NBLK else None
                )
                attn_out(blk, ets_cur)
                ets_cur = ets_next

            # ---- final projection ----
            for m in range(NBLK):
                for n2 in range(2):
                    ps = ppj.tile([P, 512], f32, tag="pp", name="pp")
                    for k in range(KD):
                        nc.tensor.matmul(
                            ps[:],
                            lhsT=outT[k][:, m * P:(m + 1) * P],
                            rhs=W["wo"][k][:, n2 * 512:(n2 + 1) * 512],
                            start=(k == 0),
                            stop=(k == KD - 1),
                        )
                    fin = finp.tile([P, 512], f32, tag="fin", name="fin")
                    if n2 == 0:
                        nc.scalar.activation(
                            fin[:], ps[:], mybir.ActivationFunctionType.Copy
                        )
                    else:
                        nc.vector.tensor_copy(fin[:], ps[:])
                    nc.sync.dma_start(
                        out_d[t0 + m * P:t0 + (m + 1) * P, n2 * 512:(n2 + 1) * 512],
                        fin[:],
                    )

    nc.compile()
    return nc


def _get_compiled():
    global _compiled
    if _compiled is None:
        _compiled = _build()
    return _compiled


def _prep_inputs(x, bias, Wq, Wk, Wv, Wo):
    bf = ml_dtypes.bfloat16
    xr = x.reshape(N_CORES, T, D)
    xts = np.ascontiguousarray(xr.transpose(0, 2, 1)).astype(bf)   # [cores, D, T]
    ws = {
        "wq": np.ascontiguousarray(Wq.astype(bf)),
        "wk": np.ascontiguousarray(Wk.astype(bf)),
        "wv": np.ascontiguousarray(Wv.astype(bf)),
        "wo": np.ascontiguousarray(Wo.astype(bf)),
    }
    eb = np.exp(bias[0].astype(np.float32))          # [h, lq, lk]
    ebt = eb.transpose(2, 0, 1).reshape(L, H * L)    # [lk, h*L + lq]
    ebt = np.ascontiguousarray(
        np.concatenate([ebt, ebt], axis=0).astype(bf)
    )                                                # [128, H*L]
    in_maps = [
        {"xt": xts[c], "expbt": ebt, **ws} for c in range(N_CORES)
    ]
    return in_maps


def kernel(x, bias, Wq, Wk, Wv, Wo, _trace=False, _trace_kwargs=None):
    from concourse.bass_utils import run_bass_kernel_spmd

    nc = _get_compiled()
    in_maps = _prep_inputs(
        np.asarray(x, dtype=np.float32),
        np.asarray(bias, dtype=np.float32),
        np.asarray(Wq, dtype=np.float32),
        np.asarray(Wk, dtype=np.float32),
        np.asarray(Wv, dtype=np.float32),
        np.asarray(Wo, dtype=np.float32),
    )
    res = run_bass_kernel_spmd(
        nc, in_maps, list(range(N_CORES)), trace=_trace, **(_trace_kwargs or {})
    )
    out = np.stack([np.asarray(res.results[c]["out"]) for c in range(N_CORES)])
    out = out.reshape(B, L, D).astype(np.float32)
    if _trace:
        return out, res
    return out


# revision 20
# speedup vs baseline: 1.2810x; 1.2587x over previous
"""ChessformerAttention Trainium2 kernel (v2).

Full-input contract: kernel(**inputs) takes the unsharded inputs
(x [256,64,1024] f32, bias [1,16,64,64] f32, Wq/Wk/Wv/Wo [1024,1024] f32)
and returns the full [256,64,1024] f32 output.

Strategy: data-parallel over batch across 8 NeuronCores (32 batches each).
Host pre-work (input-layout transforms only): shard x, pre-transpose to
[D, tokens] bf16, cast weights to bf16, precompute exp(bias) transposed.

On-device per core (all matmuls bf16, f32 accumulation):
  qT/kT = W^T-stationary projections -> [head_dim-major, tokens]
  v     = x @ Wv                     -> [tokens, head_dim-major]
  attention in 2-batch blocks (128 partitions: batch b0 on 0-63, b1 on
  64-127 via PE tile_position): scoresT = K Q^T, exp on ACT (128-wide),
  * exp(bias) on DVE, out = exp @ V with piggybacked ones-matmul
  denominators, reciprocal+broadcast multiply normalizes into a
  [tokens, D] tile, DMA-XBAR transposes to [D-major, tokens], and a
  final matmul applies Wo.
"""

import os
import numpy as np
import ml_dtypes

KTRANS = os.environ.get("KTRANS", "pe")   # "dma" | "pe"

B, L, D = 256, 64, 1024
H, HD = 16, 64
N_CORES = 8
BC = B // N_CORES            # batches per core
T = BC * L                   # tokens per core
SG = 4                       # super-groups per core
TSG = T // SG                # tokens per super-group
NBLK = TSG // 128            # 2-batch blocks per super-group
P = 128
KD = D // P                  # 128-row chunks of the model dim

_compiled = None


def _build():
    import concourse.bass as bass
    import concourse.mybir as mybir
    import concourse.tile as tile
    from concourse import bacc
    from contextlib import ExitStack

    from concourse.masks import make_identity

    bf16 = mybir.dt.bfloat16
    f32 = mybir.dt.float32
    EXP = mybir.ActivationFunctionType.Exp

    nc = bacc.Bacc(
        "TRN2",
        target_bir_lowering=False,
        debug=False,
        enable_asserts=False,
        num_devices=N_CORES,
    )
    xt_d = nc.dram_tensor("xt", [D, T], bf16, kind="ExternalInput").ap()
    w_d = {
        name: nc.dram_tensor(name, [D, D], bf16, kind="ExternalInput").ap()
        for name in ("wq", "wk", "wv", "wo")
    }
    eb_d = nc.dram_tensor("expbt", [P, H * L], bf16, kind="ExternalInput").ap()
    out_d = nc.dram_tensor("out", [T, D], f32, kind="ExternalOutput").ap()

    with tile.TileContext(nc) as tc, ExitStack() as ctx:
        const = ctx.enter_context(tc.tile_pool(name="const", bufs=1))
        wpool = ctx.enter_context(tc.tile_pool(name="w", bufs=1))
        xpool = ctx.enter_context(tc.tile_pool(name="x", bufs=1))
        qkv = ctx.enter_context(tc.tile_pool(name="qkv", bufs=2))
        etp = ctx.enter_context(tc.tile_pool(name="et", bufs=2))
        oap = ctx.enter_context(tc.tile_pool(name="oa", bufs=2))
        otp = ctx.enter_context(tc.tile_pool(name="ot", bufs=2))
        rcp = ctx.enter_context(tc.tile_pool(name="rc", bufs=2))
        finp = ctx.enter_context(tc.tile_pool(name="fin", bufs=3))
        ppj = ctx.enter_context(tc.tile_pool(name="ppj", bufs=2, space="PSUM"))
        psc = ctx.enter_context(
            tc.tile_pool(name="psc", bufs=3 if KTRANS == "dma" else 2, space="PSUM")
        )
        pou = ctx.enter_context(tc.tile_pool(name="pou", bufs=2, space="PSUM"))
        pde = ctx.enter_context(tc.tile_pool(name="pde", bufs=1, space="PSUM"))
        if KTRANS == "pe":
            ptp = ctx.enter_context(tc.tile_pool(name="ptp", bufs=1, space="PSUM"))

        ones = const.tile([P, 1], bf16, tag="ones", name="ones")
        nc.any.memset(ones[:], 1.0)
        if KTRANS == "pe":
            ident = const.tile([P, P], bf16, tag="ident", name="ident")
            make_identity(nc, ident[:])
        expbt = const.tile([P, H * L], bf16, tag="expbt", name="expbt")
        nc.sync.dma_start(expbt[:], eb_d[:])

        W = {}
        for name in ("wq", "wk", "wv", "wo"):
            W[name] = []
            for k in range(KD):
                t = wpool.tile([P, D], bf16, tag=f"{name}{k}", name=f"{name}{k}")
                nc.sync.dma_start(t[:], w_d[name][k * P:(k + 1) * P, :])
                W[name].append(t)

        xt = []
        for k in range(KD):
            t = xpool.tile([P, T], bf16, tag=f"xt{k}", name=f"xt{k}")
            nc.sync.dma_start(t[:], xt_d[k * P:(k + 1) * P, :])
            xt.append(t)



        for sg in range(SG):
            t0 = sg * TSG

            # ---- k projection -> [hn-major, tokens-of-sg], one wide tile;
            #      q projection -> two zero-padded parity variants ----
            kT = qkv.tile([P, KD, TSG], bf16, tag="kT", name="kT")
            qz = [
                qkv.tile([P, KD, TSG], bf16, tag=f"qz{par}", name=f"qz{par}")
                for par in range(2)
            ]
            nc.gpsimd.memset(qz[0][64:128, :, :], 0.0)
            nc.gpsimd.memset(qz[1][0:64, :, :], 0.0)
            for wkey in ("wq", "wk"):
                for n in range(KD):
                    ps = ppj.tile([P, TSG], f32, tag="pp", name="pp")
                    for k in range(KD):
                        nc.tensor.matmul(
                            ps[:],
                            lhsT=W[wkey][k][:, n * P:(n + 1) * P],
                            rhs=xt[k][:, t0:t0 + TSG],
                            start=(k == 0),
                            stop=(k == KD - 1),
                        )
                    if wkey == "wq":
                        nc.scalar.activation(
                            qz[0][0:64, n, :], ps[0:64, :],
                            mybir.ActivationFunctionType.Copy,
                        )
                        nc.vector.tensor_copy(qz[1][64:128, n, :], ps[64:128, :])
                    else:
                        nc.vector.tensor_copy(kT[:, n, :], ps[:])

            # ---- v projection -> [tokens, hn-major] ----
            v_sb = []
            for m in range(NBLK):
                t = qkv.tile([P, D], bf16, tag=f"v{m}", name=f"v{m}")
                for n2 in range(2):
                    ps = ppj.tile([P, 512], f32, tag="pp", name="pp")
                    for k in range(KD):
                        nc.tensor.matmul(
                            ps[:],
                            lhsT=xt[k][:, t0 + m * P:t0 + (m + 1) * P],
                            rhs=W["wv"][k][:, n2 * 512:(n2 + 1) * 512],
                            start=(k == 0),
                            stop=(k == KD - 1),
                        )
                    if n2 == 0:
                        nc.scalar.activation(
                            t[:, :512], ps[:], mybir.ActivationFunctionType.Copy
                        )
                    else:
                        nc.vector.tensor_copy(t[:, 512:], ps[:])
                v_sb.append(t)

            outT = [
                otp.tile([P, TSG], bf16, tag=f"oT{k}", name=f"oT{k}")
                for k in range(KD)
            ]

            # ---- attention: 2-batch blocks, software-pipelined ----
            # scores via zero-padded q: K=128 contraction over both heads of
            # pair n; the zeroed parity half of qz selects head 2n+par.
            def attn_scores(blk):
                tok0 = blk * P
                ets = []
                for oct in range(2):
                    ps = psc.tile([P, 512], f32, tag="ps", name="ps")
                    for j2 in range(4):
                        n = oct * 4 + j2
                        for b in range(2):
                            tb = tok0 + b * 64
                            for par in range(2):
                                j = j2 * 2 + par
                                nc.tensor.matmul(
                                    ps[b * 64:(b + 1) * 64, j * 64:(j + 1) * 64],
                                    lhsT=kT[:, n, tb:tb + 64],
                                    rhs=qz[par][:, n, tb:tb + 64],
                                    start=True,
                                    stop=True,
                                    skip_group_check=True,
                                )
                    eo = etp.tile([P, 512], bf16, tag=f"eo{oct}", name=f"eo{oct}")
                    nc.scalar.activation(eo[:], ps[:], EXP, scale=0.125)
                    et = etp.tile([P, 512], bf16, tag=f"et{oct}", name=f"et{oct}")
                    nc.vector.tensor_mul(
                        et[:], eo[:], expbt[:, oct * 512:(oct + 1) * 512]
                    )
                    ets.append(et)
                return ets

            def attn_out(blk, ets):
                tok0 = blk * P
                pd = pde.tile([P, 16], f32, tag="pd", name="pd")
                pouts = []
                for oct in range(2):
                    po = pou.tile([P, 512], f32, tag="po", name="po")
                    for j in range(8):
                        h = oct * 8 + j
                        for b in range(2):
                            sl = ets[oct][b * 64:(b + 1) * 64, j * 64:(j + 1) * 64]
                            nc.tensor.matmul(
                                po[b * 64:(b + 1) * 64, j * 64:(j + 1) * 64],
                                lhsT=sl,
                                rhs=v_sb[blk][b * 64:(b + 1) * 64, h * 64:(h + 1) * 64],
                                start=True,
                                stop=True,
                            )
                            nc.tensor.matmul(
                                pd[b * 64:(b + 1) * 64, h:h + 1],
                                lhsT=sl,
                                rhs=ones[b * 64:(b + 1) * 64, :],
                                start=True,
                                stop=True,
                                skip_group_check=True,
                            )
                    pouts.append(po)
                rc = rcp.tile([P, 16], f32, tag="rc", name="rc")
                nc.vector.reciprocal(rc[:], pd[:])
                oall = oap.tile([P, D], bf16, tag="oall", name="oall")
                for oct in range(2):
                    nc.vector.tensor_mul(
                        oall[:, oct * 512:(oct + 1) * 512].rearrange(
                            "p (h c) -> p h c", c=64
                        ),
                        pouts[oct][:].rearrange("p (h c) -> p h c", c=64),
                        rc[:, oct * 8:(oct + 1) * 8][:, :, None].broadcast_to(
                            [P, 8, 64]
                        ),
                    )
                if KTRANS == "dma":
                    for k in range(KD):
                        nc.sync.dma_start_transpose(
                            outT[k][:, tok0:tok0 + P], oall[:, k * P:(k + 1) * P]
                        )
                else:
                    for half in range(2):
                        pt = ptp.tile([P, 512], bf16, tag="pt", name="pt")
                        for k2 in range(4):
                            k = half * 4 + k2
                            nc.tensor.transpose(
                                pt[:, k2 * P:(k2 + 1) * P],
                                oall[:, k * P:(k + 1) * P],
                                ident[:],
                            )
                            if k2 % 2 == 0:
                                nc.scalar.activation(
                                    outT[k][:, tok0:tok0 + P],
                                    pt[:, k2 * P:(k2 + 1) * P],
                                    mybir.ActivationFunctionType.Copy,
                                )
                            else:
                                nc.vector.tensor_copy(
                                    outT[k][:, tok0:tok0 + P],
                                    pt[:, k2 * P:(k2 + 1) * P],
                                )

            ets_cur = attn_scores(0)
            for blk in range(NBLK):
                ets_next = attn_scores(blk + 1) if blk + 1 < NBLK else None
                attn_out(blk, ets_cur)
                ets_cur = ets_next

            # ---- final projection ----
            for m in range(NBLK):
                for n2 in range(2):
                    ps = ppj.tile([P, 512], f32, tag="pp", name="pp")
                    for k in range(KD):
                        nc.tensor.matmul(
                            ps[:],
                            lhsT=outT[k][:, m * P:(m + 1) * P],
                            rhs=W["wo"][k][:, n2 * 512:(n2 + 1) * 512],
                            start=(k == 0),
                            stop=(k == KD - 1),
                        )
                    fin = finp.tile([P, 512], f32, tag="fin", name="fin")
                    if n2 == 0:
                        nc.scalar.activation(
                            fin[:], ps[:], mybir.ActivationFunctionType.Copy
                        )
                    else:
                        nc.vector.tensor_copy(fin[:], ps[:])
                    nc.sync.dma_start(
                        out_d[t0 + m * P:t0 + (m + 1) * P, n2 * 512:(n2 + 1) * 512],
                        fin[:],
                    )

    nc.compile()
    return nc


def _get_compiled():
    global _compiled
    if _compiled is None:
        _compiled = _build()
    return _compiled


def _prep_inputs(x, bias, Wq, Wk, Wv, Wo):
    bf = ml_dtypes.bfloat16
    xr = x.reshape(N_CORES, T, D)
    xts = np.ascontiguousarray(xr.transpose(0, 2, 1)).astype(bf)   # [cores, D, T]
    ws = {
        "wq": np.ascontiguousarray(Wq.astype(bf)),
        "wk": np.ascontiguousarray(Wk.astype(bf)),
        "wv": np.ascontiguousarray(Wv.astype(bf)),
        "wo": np.ascontiguousarray(Wo.astype(bf)),
    }
    eb = np.exp(bias[0].astype(np.float32))          # [h, lq, lk]
    ebt = eb.transpose(2, 0, 1).reshape(L, H * L)    # [lk, h*L + lq]
    ebt = np.ascontiguousarray(
        np.concatenate([ebt, ebt], axis=0).astype(bf)
    )                                                # [128, H*L]
    in_maps = [
        {"xt": xts[c], "expbt": ebt, **ws} for c in range(N_CORES)
    ]
    return in_maps


def kernel(x, bias, Wq, Wk, Wv, Wo, _trace=False, _trace_kwargs=None):
    from concourse.bass_utils import run_bass_kernel_spmd

    nc = _get_compiled()
    in_maps = _prep_inputs(
        np.asarray(x, dtype=np.float32),
        np.asarray(bias, dtype=np.float32),
        np.asarray(Wq, dtype=np.float32),
        np.asarray(Wk, dtype=np.float32),
        np.asarray(Wv, dtype=np.float32),
        np.asarray(Wo, dtype=np.float32),
    )
    res = run_bass_kernel_spmd(
        nc, in_maps, list(range(N_CORES)), trace=_trace, **(_trace_kwargs or {})
    )
    out = np.stack([np.asarray(res.results[c]["out"]) for c in range(N_CORES)])
    out = out.reshape(B, L, D).astype(np.float32)
    if _trace:
        return out, res
    return out
